# revision 5
# baseline (speedup 1.0000x reference)
"""PointFeaturePropagation Trainium2 kernel.

Sharding: 8 cores = 4 batches x 2 halves of N2. Each core: 4096 queries,
full N1=2048 points, replicated (BN-folded) MLP weights.

Per-core pipeline:
  score[q,p] = 2 q.p - ||p||^2  (= ||q||^2 - dist)  via K=4 fp32 matmul
  top-3 via DVE max8 + max_index; d_k = q2 - v_k; w_k = 1/(d+1e-8) normalized
  gather pts1 rows via indirect DMA; weighted sum -> interp
  xT = [interpT | pts2T]; y1T = relu(W1'.T xT + b1'); y2T = relu(W2'.T y1T + b2')
Score matmul is plain fp32 (selection needs full mantissa); MLP is fp32r.
Output returned channel-major [256, 4096] per core, host transposes back.
"""

import numpy as np

N1, N2, C1, C2 = 2048, 8192, 256, 128
QPC = N2 // 2          # queries per core
NT = QPC // 128        # 32 query tiles per core
BN_EPS = 1e-5

_CACHE = {}


def _build_program(use_bacc=True):
    from concourse import bass, mybir
    from concourse import tile
    from concourse.masks import make_identity

    f32 = mybir.dt.float32
    f32r = mybir.dt.float32r
    u32 = mybir.dt.uint32
    AF = mybir.ActivationFunctionType

    if use_bacc:
        from concourse import bacc
        nc = bacc.Bacc()
    else:
        nc = bass.Bass()

    qT_d = nc.declare_dram_parameter("qT", [4, QPC], f32, isOutput=False)
    q2m_d = nc.declare_dram_parameter("q2m", [128, NT], f32, isOutput=False)
    rhsP_d = nc.declare_dram_parameter("rhsP", [4, N1], f32, isOutput=False)
    pts1_d = nc.declare_dram_parameter("pts1", [N1, C1], f32, isOutput=False)
    pts2T_d = nc.declare_dram_parameter("pts2T", [C2, QPC], f32r, isOutput=False)
    w1_d = nc.declare_dram_parameter("W1f", [384, 256], f32r, isOutput=False)
    w2_d = nc.declare_dram_parameter("W2f", [256, 256], f32r, isOutput=False)
    b1_d = nc.declare_dram_parameter("b1f", [128, 2], f32, isOutput=False)
    b2_d = nc.declare_dram_parameter("b2f", [128, 2], f32, isOutput=False)
    out_d = nc.declare_dram_parameter("outT", [256, QPC], f32, isOutput=True)

    with tile.TileContext(nc) as tc:
        with tc.tile_pool(name="const", bufs=1) as const, \
             tc.tile_pool(name="big", bufs=1) as big:
            qT_sb = const.tile([4, QPC], f32)
            nc.sync.dma_start(out=qT_sb, in_=qT_d[:])
            q2m_sb = const.tile([128, NT], f32)
            nc.sync.dma_start(out=q2m_sb, in_=q2m_d[:])
            rhs_sb = const.tile([4, N1], f32)
            nc.sync.dma_start(out=rhs_sb, in_=rhsP_d[:])
            w1_sb = []
            for k in range(3):
                w1k = const.tile([128, 256], f32r, name=f"w1_{k}")
                nc.sync.dma_start(out=w1k, in_=w1_d[k * 128:(k + 1) * 128, :])
                w1_sb.append(w1k)
            w2_sb = []
            for k in range(2):
                w2k = const.tile([128, 256], f32r, name=f"w2_{k}")
                nc.sync.dma_start(out=w2k, in_=w2_d[k * 128:(k + 1) * 128, :])
                w2_sb.append(w2k)
            b1_sb = const.tile([128, 2], f32)
            nc.sync.dma_start(out=b1_sb, in_=b1_d[:])
            b2_sb = const.tile([128, 2], f32)
            nc.sync.dma_start(out=b2_sb, in_=b2_d[:])
            ident = const.tile([128, 128], f32)
            make_identity(nc, ident)

            # xT = MLP input, channel-major: rows 0-255 interpT, 256-383 pts2T
            xT = [big.tile([128, QPC], f32r, name=f"xT{i}") for i in range(3)]
            nc.sync.dma_start(out=xT[2], in_=pts2T_d[:])
            y1T = [big.tile([128, QPC], f32r, name=f"y1T{i}") for i in range(2)]

            # ---------------- Phase 1: KNN + interp ----------------
            with tc.tile_pool(name="p1", bufs=2) as p1, \
                 tc.tile_pool(name="sc", bufs=2) as sc_pool, \
                 tc.tile_pool(name="ps_s", bufs=3, space="PSUM") as ps_pool, \
                 tc.tile_pool(name="ps_t", bufs=2, space="PSUM") as pt_pool:
                for t in range(NT):
                    qs = slice(t * 128, (t + 1) * 128)
                    score = sc_pool.tile([128, N1], f32, name="score")
                    lhs = qT_sb[:, qs]
                    for j in range(4):
                        js = slice(j * 512, (j + 1) * 512)
                        ps = ps_pool.tile([128, 512], f32, name="ps")
                        nc.tensor.matmul(ps, lhsT=lhs, rhs=rhs_sb[:, js],
                                         start=True, stop=True)
                        nc.scalar.activation(score[:, js], ps, AF.Copy)

                    v8 = p1.tile([128, 8], f32, name="v8")
                    nc.vector.max(v8, score)
                    i8 = p1.tile([128, 8], u32, name="i8")
                    nc.vector.max_index(i8, v8, score)

                    # d_r = relu(q2 - 1e-10 - v);  w' = 1/(d_r + 1.01e-8)
                    dr = p1.tile([128, 3], f32, name="dr")
                    nc.scalar.activation(dr, v8[:, 0:3], AF.Relu,
                                         bias=q2m_sb[:, t:t + 1], scale=-1.0)
                    dq = p1.tile([128, 3], f32, name="dq")
                    nc.scalar.activation(dq, dr, AF.Copy, bias=1.01e-8)
                    wr = p1.tile([128, 3], f32, name="wr")
                    nc.vector.reciprocal(wr, dq)
                    wcp = p1.tile([128, 3], f32, name="wcp")
                    sw = p1.tile([128, 1], f32, name="sw")
                    nc.scalar.activation(wcp, wr, AF.Copy, accum_out=sw)
                    rs = p1.tile([128, 1], f32, name="rs")
                    nc.vector.reciprocal(rs, sw)
                    wn = p1.tile([128, 3], f32, name="wn")
                    nc.scalar.activation(wn, wr, AF.Copy, scale=rs[:, 0:1])

                    g = []
                    for k in range(3):
                        gk = p1.tile([128, C1], f32, name=f"g{k}")
                        nc.gpsimd.indirect_dma_start(
                            out=gk, out_offset=None, in_=pts1_d[:],
                            in_offset=bass.IndirectOffsetOnAxis(
                                ap=i8[:, k:k + 1], axis=0))
                        g.append(gk)

                    acc0 = p1.tile([128, C1], f32, name="acc0")
                    nc.vector.tensor_scalar_mul(acc0, g[0], wn[:, 0:1])
                    acc1 = p1.tile([128, C1], f32, name="acc1")
                    nc.vector.scalar_tensor_tensor(
                        acc1, in0=g[1], scalar=wn[:, 1:2], in1=acc0,
                        op0=mybir.AluOpType.mult, op1=mybir.AluOpType.add)
                    interp = p1.tile([128, C1], f32, name="interp")
                    nc.vector.scalar_tensor_tensor(
                        interp, in0=g[2], scalar=wn[:, 2:3], in1=acc1,
                        op0=mybir.AluOpType.mult, op1=mybir.AluOpType.add)

                    ptp = pt_pool.tile([128, 256], f32, name="ptp")
                    for cchunk in range(2):
                        cs = slice(cchunk * 128, (cchunk + 1) * 128)
                        nc.tensor.transpose(ptp[:, cs], interp[:, cs], ident)
                        nc.scalar.activation(xT[cchunk][:, qs], ptp[:, cs],
                                             AF.Copy)

            # ---------------- Phase 2: MLP ----------------
            with tc.tile_pool(name="p2", bufs=2) as p2, \
                 tc.tile_pool(name="ps_m", bufs=2, space="PSUM") as pm_pool:
                for c in range(QPC // 512):
                    cs = slice(c * 512, (c + 1) * 512)
                    for m in range(2):
                        ms = slice(m * 128, (m + 1) * 128)
                        pm = pm_pool.tile([128, 512], f32, name="pm1")
                        for k in range(3):
                            nc.tensor.matmul(
                                pm, lhsT=w1_sb[k][:, ms],
                                rhs=xT[k][:, cs],
                                start=(k == 0), stop=(k == 2))
                        nc.scalar.activation(y1T[m][:, cs], pm, AF.Relu,
                                             bias=b1_sb[:, m:m + 1])
                    for m in range(2):
                        ms = slice(m * 128, (m + 1) * 128)
                        pm2 = pm_pool.tile([128, 512], f32, name="pm2")
                        for k in range(2):
                            nc.tensor.matmul(
                                pm2, lhsT=w2_sb[k][:, ms],
                                rhs=y1T[k][:, cs],
                                start=(k == 0), stop=(k == 1))
                        ys = p2.tile([128, 512], f32, name="ys")
                        nc.scalar.activation(ys, pm2, AF.Relu,
                                             bias=b2_sb[:, m:m + 1])
                        nc.sync.dma_start(out=out_d[ms, cs], in_=ys)

    return nc


def _prep_core_inputs(core, xyz1, xyz2, pts1, pts2, W1f, W2f, b1f, b2f):
    b, h = core // 2, core % 2
    qs = slice(h * QPC, (h + 1) * QPC)
    q = xyz2[b, qs]                      # [4096, 3]
    qT = np.empty((4, QPC), np.float32)
    qT[0:3] = (2.0 * q).T
    qT[3] = -1.0
    q2 = np.sum(q * q, axis=-1, dtype=np.float32)
    q2m = np.ascontiguousarray(q2.reshape(NT, 128).T) - np.float32(1e-10)
    p = xyz1[b]                          # [2048, 3]
    rhsP = np.empty((4, N1), np.float32)
    rhsP[0:3] = p.T
    rhsP[3] = np.sum(p * p, axis=-1, dtype=np.float32)
    return {
        "qT": qT,
        "q2m": np.ascontiguousarray(q2m, dtype=np.float32),
        "rhsP": rhsP,
        "pts1": np.ascontiguousarray(pts1[b]),
        "pts2T": np.ascontiguousarray(pts2[b, qs].T),
        "W1f": W1f, "W2f": W2f, "b1f": b1f, "b2f": b2f,
    }


def kernel(xyz1, xyz2, pts1, pts2, W1, b1, g1, be1, rm1, rv1,
           W2, b2, g2, be2, rm2, rv2):
    from concourse.bass_utils import run_bass_kernel_spmd

    if "nc" not in _CACHE:
        nc = _build_program()
        nc.finalize()
        _CACHE["nc"] = nc
    nc = _CACHE["nc"]

    a1 = g1 / np.sqrt(rv1 + BN_EPS)
    W1f = (W1 * a1[None, :]).astype(np.float32)
    b1f = (((b1 - rm1) * a1 + be1).astype(np.float32)
           .reshape(2, 128).T.copy())
    a2 = g2 / np.sqrt(rv2 + BN_EPS)
    W2f = (W2 * a2[None, :]).astype(np.float32)
    b2f = (((b2 - rm2) * a2 + be2).astype(np.float32)
           .reshape(2, 128).T.copy())

    in_maps = [
        _prep_core_inputs(c, xyz1, xyz2, pts1, pts2, W1f, W2f, b1f, b2f)
        for c in range(8)
    ]
    res = run_bass_kernel_spmd(nc, in_maps, core_ids=list(range(8)))
    _CACHE["res"] = res

    out = np.empty((4, N2, 256), np.float32)
    for c in range(8):
        b, h = c // 2, c % 2
        out[b, h * QPC:(h + 1) * QPC, :] = res.results[c]["outT"].T
    return out


# revision 31
# speedup vs baseline: 1.2787x; 1.2787x over previous
"""PointFeaturePropagation Trainium2 kernel.

Sharding: 8 cores = 4 batches x 2 halves of N2. Each core: 4096 queries,
full N1=2048 points, replicated (BN-folded) MLP weights.

Per-core pipeline:
  score[q,p] = 2 q.p - ||p||^2  (= ||q||^2 - dist)  via K=4 fp32 matmul
  top-3 via DVE max8 + max_index; d_k = q2 - v_k; w_k = 1/(d+1e-8) normalized
  gather pts1 rows via indirect DMA; weighted sum -> interp
  xT = [interpT | pts2T]; y1T = relu(W1'.T xT + b1'); y2T = relu(W2'.T y1T + b2')
Score matmul is plain fp32 (selection needs full mantissa); MLP is fp32r.
Output returned channel-major [256, 4096] per core, host transposes back.

v4: PSUM->SBUF evac all on ACT (GPSIMD may not touch PSUM) as 2x[128,1024];
wsum on Pool; chunked qT/rhsP/pts2T loads spread over SP/Pool/ACT DMA queues
to cut startup; MLP interleaved every 2 tiles (256-col chunks) to cut tail.
"""

import numpy as np

N1, N2, C1, C2 = 2048, 8192, 256, 128
QPC = N2 // 2          # queries per core
NT = QPC // 128        # 32 query tiles per core
BN_EPS = 1e-5

_CACHE = {}


def _build_program(use_bacc=True):
    from concourse import bass, mybir
    from concourse import tile
    from concourse.masks import make_identity

    f32 = mybir.dt.float32
    f32r = mybir.dt.float32r
    u32 = mybir.dt.uint32
    AF = mybir.ActivationFunctionType

    if use_bacc:
        from concourse import bacc
        nc = bacc.Bacc()
    else:
        nc = bass.Bass()

    qT_d = nc.declare_dram_parameter("qT", [4, QPC], f32, isOutput=False)
    q2m_d = nc.declare_dram_parameter("q2m", [128, NT], f32, isOutput=False)
    rhsP_d = nc.declare_dram_parameter("rhsP", [4, N1], f32, isOutput=False)
    pts1_d = nc.declare_dram_parameter("pts1", [N1, C1], f32, isOutput=False)
    pts2T_d = nc.declare_dram_parameter("pts2T", [C2, QPC], f32r, isOutput=False)
    w1_d = nc.declare_dram_parameter("W1f", [384, 256], f32r, isOutput=False)
    w2_d = nc.declare_dram_parameter("W2f", [256, 256], f32r, isOutput=False)
    b1_d = nc.declare_dram_parameter("b1f", [128, 2], f32, isOutput=False)
    b2_d = nc.declare_dram_parameter("b2f", [128, 2], f32, isOutput=False)
    out_d = nc.declare_dram_parameter("outT", [256, QPC], f32, isOutput=True)

    with tile.TileContext(nc) as tc:
        with tc.tile_pool(name="const", bufs=1) as const, \
             tc.tile_pool(name="big", bufs=1) as big:
            # Spread first-needed loads over the three DMA queues
            # (SP/ACT/Pool) so the first matmuls can start ~2us in.
            qT_sb = const.tile([4, QPC], f32)
            rhs_sb = const.tile([4, N1], f32)
            nc.sync.dma_start(out=qT_sb[:, 0:512], in_=qT_d[:, 0:512])
            nc.scalar.dma_start(out=rhs_sb[:, 0:512], in_=rhsP_d[:, 0:512])
            nc.gpsimd.dma_start(out=rhs_sb[:, 512:1024],
                                in_=rhsP_d[:, 512:1024])
            nc.gpsimd.dma_start(out=rhs_sb[:, 1024:1536],
                                in_=rhsP_d[:, 1024:1536])
            q2m_sb = const.tile([128, NT], f32)
            nc.sync.dma_start(out=q2m_sb, in_=q2m_d[:])
            nc.sync.dma_start(out=rhs_sb[:, 1536:2048],
                              in_=rhsP_d[:, 1536:2048])
            for cc in range(1, 8):
                cs = slice(cc * 512, (cc + 1) * 512)
                nc.sync.dma_start(out=qT_sb[:, cs], in_=qT_d[:, cs])
            scratch = const.tile([128, 128], f32, name="scratch")
            nc.gpsimd.memset(scratch, 0.0)
            # MLP weight tiles; their DMAs are emitted at the end of
            # tile 0 so they don't delay the first score evacuations
            # on the ACT queue (first use is the t=1 MLP chunk).
            w1_sb = [const.tile([128, 256], f32r, name=f"w1_{k}")
                     for k in range(3)]
            w2_sb = [const.tile([128, 256], f32r, name=f"w2_{k}")
                     for k in range(2)]
            b1_sb = const.tile([128, 2], f32)
            b2_sb = const.tile([128, 2], f32)
            ident = const.tile([128, 128], f32)
            make_identity(nc, ident)

            # xT = MLP input, channel-major: rows 0-255 interpT, 256-383 pts2T
            xT = [big.tile([128, QPC], f32r, name=f"xT{i}") for i in range(3)]
            y1T = [big.tile([128, QPC], f32r, name=f"y1T{i}") for i in range(2)]

            # -------- KNN + interp, MLP chunk interleaved every 2 tiles -----
            with tc.tile_pool(name="p1", bufs=2) as p1, \
                 tc.tile_pool(name="sc", bufs=2) as sc_pool, \
                 tc.tile_pool(name="ps_s", bufs=2, space="PSUM") as ps_pool, \
                 tc.tile_pool(name="ps_t", bufs=2, space="PSUM") as pt_pool, \
                 tc.tile_pool(name="ps_m", bufs=1, space="PSUM") as pm_pool:
                # Pre-warm: the PE clock ramps to full speed only after
                # ~3us of continuous use, and ACT pays a one-time
                # activation-table load.  Burn both while the first DMAs
                # are still in flight so the real work runs at full rate.
                wps = ps_pool.tile([128, 1024], f32, name="ps")
                for _ in range(6):
                    nc.tensor.matmul(wps[:, 0:128], lhsT=scratch,
                                     rhs=scratch, start=True, stop=True)
                wact = const.tile([128, 1], f32, name="wact")
                nc.scalar.activation(wact, scratch[:, 0:1], AF.Copy)
                for t in range(NT):
                    qs = slice(t * 128, (t + 1) * 128)
                    if t < 16:
                        # pts2T chunk t streams in on the ACT queue
                        ts2 = slice(t * 256, (t + 1) * 256)
                        nc.scalar.dma_start(out=xT[2][:, ts2],
                                            in_=pts2T_d[:, ts2])
                    score = sc_pool.tile([128, N1], f32, name="score")
                    lhs = qT_sb[:, qs]
                    for half in range(2):
                        ps = ps_pool.tile([128, 1024], f32, name="ps")
                        for j2 in range(2):
                            j = half * 2 + j2
                            nc.tensor.matmul(
                                ps[:, j2 * 512:(j2 + 1) * 512], lhsT=lhs,
                                rhs=rhs_sb[:, j * 512:(j + 1) * 512],
                                start=True, stop=True)
                        hs = slice(half * 1024, (half + 1) * 1024)
                        nc.scalar.activation(score[:, hs], ps, AF.Copy)

                    v8 = p1.tile([128, 8], f32, name="v8")
                    nc.vector.max(v8, score)
                    i8 = p1.tile([128, 8], u32, name="i8")
                    nc.vector.max_index(i8, v8, score)

                    # ndq = min(v - q2m, -5e-7) = -(d + 1e-8), clamped away
                    # from 0.  w_k = (1/d_k)/sum(1/d_j) = u_k/sum(u_j) with
                    # u_k = prod of the other two (negated) dists, so the
                    # whole weight chain runs on Pool with no DVE recips.
                    ndq = p1.tile([128, 3], f32, name="ndq")
                    nc.gpsimd.tensor_scalar(
                        out=ndq, in0=v8[:, 0:3], scalar1=q2m_sb[:, t:t + 1],
                        scalar2=-5e-7, op0=mybir.AluOpType.subtract,
                        op1=mybir.AluOpType.min)
                    u = p1.tile([128, 3], f32, name="u")
                    nc.gpsimd.tensor_mul(u[:, 0:1], ndq[:, 1:2], ndq[:, 2:3])
                    nc.gpsimd.tensor_mul(u[:, 1:2], ndq[:, 0:1], ndq[:, 2:3])
                    nc.gpsimd.tensor_mul(u[:, 2:3], ndq[:, 0:1], ndq[:, 1:2])
                    sw0 = p1.tile([128, 1], f32, name="sw0")
                    nc.gpsimd.tensor_add(sw0, u[:, 0:1], u[:, 1:2])
                    sw = p1.tile([128, 1], f32, name="sw")
                    nc.gpsimd.tensor_add(sw, sw0, u[:, 2:3])
                    rs = p1.tile([128, 1], f32, name="rs")
                    nc.vector.reciprocal(rs, sw)
                    wn = p1.tile([128, 3], f32, name="wn")
                    nc.gpsimd.tensor_scalar_mul(wn, u, rs[:, 0:1])

                    g = []
                    for k in range(3):
                        gk = p1.tile([128, C1], f32, name=f"g{k}")
                        nc.gpsimd.indirect_dma_start(
                            out=gk, out_offset=None, in_=pts1_d[:],
                            in_offset=bass.IndirectOffsetOnAxis(
                                ap=i8[:, k:k + 1], axis=0))
                        g.append(gk)

                    wg = []
                    for k in range(3):
                        wk = p1.tile([128, C1], f32, name=f"wg{k}")
                        nc.gpsimd.tensor_scalar_mul(wk, g[k], wn[:, k:k + 1])
                        wg.append(wk)
                    acc1 = p1.tile([128, C1], f32, name="acc1")
                    nc.gpsimd.tensor_add(acc1, wg[0], wg[1])
                    interp = p1.tile([128, C1], f32, name="interp")
                    nc.gpsimd.tensor_add(interp, acc1, wg[2])

                    ptp = pt_pool.tile([128, 256], f32, name="ptp")
                    for cchunk in range(2):
                        cs = slice(cchunk * 128, (cchunk + 1) * 128)
                        nc.tensor.transpose(ptp[:, cs], interp[:, cs], ident)
                        nc.scalar.activation(xT[cchunk][:, qs], ptp[:, cs],
                                             AF.Copy)

                    if t == 0:
                        for k in range(3):
                            nc.scalar.dma_start(
                                out=w1_sb[k],
                                in_=w1_d[k * 128:(k + 1) * 128, :])
                        for k in range(2):
                            nc.scalar.dma_start(
                                out=w2_sb[k],
                                in_=w2_d[k * 128:(k + 1) * 128, :])
                        nc.scalar.dma_start(out=b1_sb, in_=b1_d[:])
                        nc.scalar.dma_start(out=b2_sb, in_=b2_d[:])

                    # MLP chunks: 256 cols every 2 tiles (f32r matmuls
                    # need >=256 moving cols for 1 cyc/row).
                    mcs = None
                    if t % 2 == 1:
                        c = t // 2
                        mcs = slice(c * 256, (c + 1) * 256)
                    if mcs is not None:
                        w = mcs.stop - mcs.start
                        for m in range(2):
                            ms = slice(m * 128, (m + 1) * 128)
                            pm = pm_pool.tile([128, 256], f32, name="pm1")
                            for k in range(3):
                                nc.tensor.matmul(
                                    pm[:, 0:w], lhsT=w1_sb[k][:, ms],
                                    rhs=xT[k][:, mcs],
                                    start=(k == 0), stop=(k == 2))
                            nc.scalar.activation(y1T[m][:, mcs], pm[:, 0:w],
                                                 AF.Relu,
                                                 bias=b1_sb[:, m:m + 1])
                        for m in range(2):
                            ms = slice(m * 128, (m + 1) * 128)
                            pm2 = pm_pool.tile([128, 256], f32, name="pm2")
                            for k in range(2):
                                nc.tensor.matmul(
                                    pm2[:, 0:w], lhsT=w2_sb[k][:, ms],
                                    rhs=y1T[k][:, mcs],
                                    start=(k == 0), stop=(k == 1))
                            ys = p1.tile([128, 256], f32, name="ys")
                            nc.scalar.activation(ys[:, 0:w], pm2[:, 0:w],
                                                 AF.Relu,
                                                 bias=b2_sb[:, m:m + 1])
                            nc.sync.dma_start(out=out_d[ms, mcs],
                                              in_=ys[:, 0:w])

    return nc


def _prep_core_inputs(core, xyz1, xyz2, pts1, pts2, W1f, W2f, b1f, b2f):
    b, h = core // 2, core % 2
    qs = slice(h * QPC, (h + 1) * QPC)
    q = xyz2[b, qs]                      # [4096, 3]
    qT = np.empty((4, QPC), np.float32)
    qT[0:3] = (2.0 * q).T
    qT[3] = -1.0
    q2 = np.sum(q * q, axis=-1, dtype=np.float32)
    q2m = (np.ascontiguousarray(q2.reshape(NT, 128).T)
           - np.float32(1e-10) + np.float32(1.01e-8))
    p = xyz1[b]                          # [2048, 3]
    rhsP = np.empty((4, N1), np.float32)
    rhsP[0:3] = p.T
    rhsP[3] = np.sum(p * p, axis=-1, dtype=np.float32)
    return {
        "qT": qT,
        "q2m": np.ascontiguousarray(q2m, dtype=np.float32),
        "rhsP": rhsP,
        "pts1": np.ascontiguousarray(pts1[b]),
        "pts2T": np.ascontiguousarray(pts2[b, qs].T),
        "W1f": W1f, "W2f": W2f, "b1f": b1f, "b2f": b2f,
    }


def kernel(xyz1, xyz2, pts1, pts2, W1, b1, g1, be1, rm1, rv1,
           W2, b2, g2, be2, rm2, rv2):
    from concourse.bass_utils import run_bass_kernel_spmd

    if "nc" not in _CACHE:
        nc = _build_program()
        nc.finalize()
        _CACHE["nc"] = nc
    nc = _CACHE["nc"]

    a1 = g1 / np.sqrt(rv1 + BN_EPS)
    W1f = (W1 * a1[None, :]).astype(np.float32)
    b1f = (((b1 - rm1) * a1 + be1).astype(np.float32)
           .reshape(2, 128).T.copy())
    a2 = g2 / np.sqrt(rv2 + BN_EPS)
    W2f = (W2 * a2[None, :]).astype(np.float32)
    b2f = (((b2 - rm2) * a2 + be2).astype(np.float32)
           .reshape(2, 128).T.copy())

    in_maps = [
        _prep_core_inputs(c, xyz1, xyz2, pts1, pts2, W1f, W2f, b1f, b2f)
        for c in range(8)
    ]
    res = run_bass_kernel_spmd(nc, in_maps, core_ids=list(range(8)))
    _CACHE["res"] = res

    out = np.empty((4, N2, 256), np.float32)
    for c in range(8):
        b, h = c // 2, c % 2
        out[b, h * QPC:(h + 1) * QPC, :] = res.results[c]["outT"].T
    return out
